# revision 14
# baseline (speedup 1.0000x reference)
"""Mixture-of-Experts (8 experts, top-2, D=1024, H=2048, T=8192) on 8 trn2 cores.

Strategy: expert-parallel with host-side routing, load-balanced by splitting
each expert's FFN along the hidden dim into two half-H jobs (16 jobs total).
  - Router (tiny: [T,D]@[D,E]) runs on host in float64; top-2 selection was
    verified to match fp32 jax (cpu + neuron) selection for this problem size.
  - Each job (expert e, half h) computes, for the tokens routed to e:
        h1T = w1[e,hH].T-chunks @ xT    [H/2, C]
        hT  = silu(h1T) * h3T           [H/2, C]  (bf16)
        yT  = (hT.T chunks) @ w2[e,:,hH].T  -> partial y [C, D], cw-scaled
    The two halves' partial y are summed on the host (linear in cw).
  - Jobs are sorted by token load and paired big-with-small onto the 8
    cores, so per-core capacity is J1+J2 ~ 2*mean instead of 2*max.
  - Activations flow in transposed (feature-major) layout so the kernel
    needs no on-device transposes; phase B puts tokens on partitions so the
    per-token combine weight is a per-partition tensor_scalar op.
"""

import sys
import types
from contextlib import ExitStack

import ml_dtypes
import numpy as np

import concourse.bass as bass
import concourse.tile as tile
from concourse import bacc, mybir
from concourse.bass_utils import run_bass_kernel_spmd


def install_axon_hooks_shim():
    """The container's antenv stub lacks axon_hooks, which
    run_bass_kernel_spmd imports whenever tracing is requested (including
    via the BASS_TRACE env var). Recreate it and register the NTFF
    profiling hook if the axon PJRT .so is present."""
    try:
        import antenv
    except ImportError:
        return False
    if "antenv.axon_hooks" in sys.modules:
        return sys.modules["antenv.axon_hooks"]._hook is not None
    mod = types.ModuleType("antenv.axon_hooks")
    mod._hook = None
    mod.set_axon_ntff_profile_hook = lambda h: setattr(mod, "_hook", h)
    mod.get_axon_ntff_profile_hook = lambda: mod._hook
    sys.modules["antenv.axon_hooks"] = mod
    antenv.axon_hooks = mod
    try:
        from trn_agent_boot.trn_boot import _ntff_profile_via_ctypes

        mod.set_axon_ntff_profile_hook(
            _ntff_profile_via_ctypes("/opt/axon/libaxon_pjrt.so")
        )
    except Exception:
        pass
    return mod._hook is not None


install_axon_hooks_shim()

E = 8  # experts
D = 1024
H = 2048
HH = H // 2  # hidden half per job
TOP_K = 2
KA = D // 128  # contraction chunks for matmul 1
KM = HH // 128  # contraction chunks for matmul 2 (half hidden)

BF16 = mybir.dt.bfloat16
F32 = mybir.dt.float32

# w1/w3 half-H piece sizes in m-chunks (small first so early matmuls start
# early)
PIECES = (1, 1, 2, 4)

_CACHE: dict[tuple, object] = {}


def _route(x2d: np.ndarray, router_w: np.ndarray):
    """Float64 router. Returns per-expert token lists, per-expert combine
    weights, and for each token its (expert, slot-in-expert-batch) pairs."""
    T = x2d.shape[0]
    logits = x2d.astype(np.float64) @ router_w.astype(np.float64).T  # [T, E]
    order = np.argsort(-logits, axis=1, kind="stable")
    top2 = order[:, :TOP_K]  # [T, 2]
    lt = np.take_along_axis(logits, top2, axis=1)
    m = lt.max(axis=1, keepdims=True)
    ex = np.exp(lt - m)
    cw = (ex / ex.sum(axis=1, keepdims=True)).astype(np.float32)  # [T, 2]

    rows = []  # rows[e]: token ids routed to expert e (ascending)
    cw_e = []  # cw_e[e]: combine weight per routed token
    slot = np.empty((T, TOP_K), np.int64)  # slot[t, k]: row of t in expert batch
    for e in range(E):
        r = np.where((top2[:, 0] == e) | (top2[:, 1] == e))[0]
        k = np.where(top2[r, 0] == e, 0, 1)
        rows.append(r)
        cw_e.append(cw[r, k])
        slot[r, k] = np.arange(len(r))
    return rows, cw_e, top2, slot


def _blocks_for(C):
    """Token blocks for a slot of capacity C (a multiple of 128). Blocks are
    3 or 4 subtiles (384/512) so phase-A matmul free dims stay well above the
    ~240-cycle LDWEIGHTS latency."""
    n = C // 128
    sizes = []
    while n > 0:
        take = 3 if n == 5 else min(4, n)
        sizes.append(take * 128)
        n -= take
    blocks, t0 = [], 0
    for sz in sizes:
        blocks.append((t0, sz))
        t0 += sz
    return blocks


def _build(caps: tuple):
    """Build + compile the per-core Bass program for slot capacities `caps`
    (each a multiple of 128).

    All inputs are shipped pre-arranged in SBUF partition-major layout so
    every DMA is ~128 large contiguous descriptors (descriptor rate, not
    bandwidth, limits small strided transfers)."""
    nslots = len(caps)
    for C in caps:
        assert C % 128 == 0
    blocks_s = [_blocks_for(C) for C in caps]

    nc = bacc.Bacc("TRN2", target_bir_lowering=False, debug=False)

    xtps, w1ps, w3ps, w2ps, cwts, ys, yvs = [], [], [], [], [], [], []
    for s, C in enumerate(caps):
        nsub = C // 128
        # x per token block, exact-size so the DMA is 128 contiguous rows
        xtps.append([
            nc.declare_dram_parameter(f"xtp{s}_{b}", [128, KA, tb], BF16, isOutput=False)
            for b, (t0, tb) in enumerate(blocks_s[s])
        ])
        w1ps.append([
            nc.declare_dram_parameter(f"w1p{s}_{p}", [128, KA, sz * 128], BF16, isOutput=False)
            for p, sz in enumerate(PIECES)
        ])
        w3ps.append([
            nc.declare_dram_parameter(f"w3p{s}_{p}", [128, KA, sz * 128], BF16, isOutput=False)
            for p, sz in enumerate(PIECES)
        ])
        w2ps.append([
            nc.declare_dram_parameter(f"w2p{s}_{dh}", [128, KM, 512], BF16, isOutput=False)
            for dh in range(2)
        ])
        cwts.append(nc.declare_dram_parameter(f"cwt{s}", [128, nsub], F32, isOutput=False))
        y = nc.declare_dram_parameter(f"y{s}", [C, D], F32, isOutput=True)
        ys.append(y)
        yvs.append(y.rearrange("(n p) d -> n p d", p=128))  # [nsub, 128, D]

    with ExitStack() as ctx:
        tc = ctx.enter_context(tile.TileContext(nc))
        wpool = ctx.enter_context(tc.tile_pool(name="weights", bufs=1))
        xpool = ctx.enter_context(tc.tile_pool(name="x", bufs=6))
        hpool = ctx.enter_context(tc.tile_pool(name="h", bufs=2))
        spool = ctx.enter_context(tc.tile_pool(name="s", bufs=3))
        ypool = ctx.enter_context(tc.tile_pool(name="y", bufs=4))
        ppool = ctx.enter_context(tc.tile_pool(name="psum", bufs=2, space="PSUM"))

        def xts_load(s, bi):
            xa = xpool.tile([128, KA, blocks_s[s][bi][1]], BF16, tag="xts")
            nc.sync.dma_start(xa[:], xtps[s][bi][:])
            return xa

        def wpiece_load(kind, src, s, p, sz):
            t = wpool.tile([128, KA, sz * 128], BF16, tag=f"{kind}{s}_{p}")
            nc.sync.dma_start(t[:], src[:])
            return t

        # DMA issue order is the DMA *execution* order; the aggregate rate is
        # ~0.36 MB/us, so interleave slot0's x blocks with its weight pieces
        # to keep the PE fed during the first ~50us.
        xts_tiles = {}
        w1p_s, w3p_s, w2p_s, cws_s = [], [], [], []

        # each dma_start costs ~0.6us of Sync-engine enqueue time, so the ramp
        # uses few, large transfers. Order = DMA execution order: the m=0
        # w1/w3 piece, x block0, then the remaining w1/w3 pieces (phase A
        # consumes them within ~15us; the later x blocks can wait)
        w1p0, w3p0 = [], []
        for p, sz in enumerate(PIECES):
            t1 = wpiece_load("w1s", w1ps[0][p], 0, p, sz)
            t3 = wpiece_load("w3s", w3ps[0][p], 0, p, sz)
            for i in range(sz):
                w1p0.append((t1, i))
                w3p0.append((t3, i))
            if p == 0:
                xts_tiles[(0, 0)] = xts_load(0, 0)
        for bi in range(1, len(blocks_s[0])):
            xts_tiles[(0, bi)] = xts_load(0, bi)
        w1p_s.append(w1p0)
        w3p_s.append(w3p0)
        row = []
        for dh in range(2):
            t2 = wpool.tile([128, KM, 512], BF16, tag=f"w2s0_{dh}")
            nc.sync.dma_start(t2[:], w2ps[0][dh][:])
            row.append(t2)
        w2p_s.append(row)
        cws = wpool.tile([128, caps[0] // 128], F32, tag="cws0")
        nc.sync.dma_start(cws[:], cwts[0][:])
        cws_s.append(cws)

        for s in range(1, nslots):
            w1p, w3p = [], []
            for p, sz in enumerate(PIECES):
                t1 = wpiece_load("w1s", w1ps[s][p], s, p, sz)
                t3 = wpiece_load("w3s", w3ps[s][p], s, p, sz)
                for i in range(sz):
                    w1p.append((t1, i))
                    w3p.append((t3, i))
            w1p_s.append(w1p)
            w3p_s.append(w3p)
            row = []
            for dh in range(2):
                t2 = wpool.tile([128, KM, 512], BF16, tag=f"w2s{s}_{dh}")
                nc.sync.dma_start(t2[:], w2ps[s][dh][:])
                row.append(t2)
            w2p_s.append(row)
            cws = wpool.tile([128, caps[s] // 128], F32, tag=f"cws{s}")
            nc.sync.dma_start(cws[:], cwts[s][:])
            cws_s.append(cws)

        def phase_a(s, bi):
            t0, tb = blocks_s[s][bi]
            xts = xts_tiles.pop((s, bi), None)
            if xts is None:
                xts = xts_load(s, bi)
            w1p, w3p = w1p_s[s], w3p_s[s]
            hts = hpool.tile([128, KM, tb], BF16, tag="hts")
            for m in range(KM):
                w1s, o1 = w1p[m]
                w3s, o3 = w3p[m]
                ph1 = ppool.tile([128, tb], F32, tag="ph1")
                for a in range(KA):
                    nc.tensor.matmul(
                        ph1[:],
                        w1s[:, a, bass.ts(o1, 128)],
                        xts[:, a, :],
                        start=(a == 0),
                        stop=(a == KA - 1),
                    )
                ph3 = ppool.tile([128, tb], F32, tag="ph3")
                for a in range(KA):
                    nc.tensor.matmul(
                        ph3[:],
                        w3s[:, a, bass.ts(o3, 128)],
                        xts[:, a, :],
                        start=(a == 0),
                        stop=(a == KA - 1),
                    )
                sil = spool.tile([128, tb], BF16, tag="sil")
                nc.scalar.activation(
                    sil[:], ph1[:], mybir.ActivationFunctionType.Silu
                )
                nc.vector.tensor_mul(hts[:, m, :], sil[:], ph3[:])
            return hts

        def phase_b(s, bi, hts, is_last):
            t0, tb = blocks_s[s][bi]
            w2p, cws = w2p_s[s], cws_s[s]
            for n in range(tb // 128):
                nsl = bass.ts(n, 128)
                gn = t0 // 128 + n  # global subtile index
                split = is_last and n == tb // 128 - 1
                if split:
                    # final subtile: emit progressively narrower pieces, each
                    # DMA'd as soon as ready, so the tail after the last
                    # matmul is a single small scale+DMA
                    pieces = [(0, 512, "py0", "ys0"), (512, 384, "py1", "ys1"),
                              (896, 128, "py1", "ys1")]
                    for d0, dw, ptag, ytag in pieces:
                        py = ppool.tile([128, dw], F32, tag=ptag)
                        for m in range(KM):
                            nc.tensor.matmul(
                                py[:],
                                hts[:, m, nsl],
                                w2p[d0 // 512][:, m, d0 % 512 : d0 % 512 + dw],
                                start=(m == 0),
                                stop=(m == KM - 1),
                            )
                        yt = ypool.tile([128, dw], F32, tag=ytag)
                        nc.vector.tensor_scalar_mul(yt[:], py[:], cws[:, gn : gn + 1])
                        nc.sync.dma_start(yvs[s][gn][:, d0 : d0 + dw], yt[:])
                    continue
                py0 = ppool.tile([128, 512], F32, tag="py0")
                py1 = ppool.tile([128, 512], F32, tag="py1")
                for m in range(KM):
                    nc.tensor.matmul(
                        py0[:],
                        hts[:, m, nsl],
                        w2p[0][:, m, :],
                        start=(m == 0),
                        stop=(m == KM - 1),
                    )
                    nc.tensor.matmul(
                        py1[:],
                        hts[:, m, nsl],
                        w2p[1][:, m, :],
                        start=(m == 0),
                        stop=(m == KM - 1),
                    )
                ys0 = ypool.tile([128, 512], F32, tag="ys0")
                nc.vector.tensor_scalar_mul(ys0[:], py0[:], cws[:, gn : gn + 1])
                nc.sync.dma_start(yvs[s][gn][:, 0:512], ys0[:])
                ys1 = ypool.tile([128, 512], F32, tag="ys1")
                nc.vector.tensor_scalar_mul(ys1[:], py1[:], cws[:, gn : gn + 1])
                nc.sync.dma_start(yvs[s][gn][:, 512:1024], ys1[:])

        # software pipeline: A(g+1) is issued before B(g) so the PE never
        # waits on w2 / output machinery during the DMA-heavy ramp
        sched = [(s, bi) for s in range(nslots) for bi in range(len(blocks_s[s]))]
        pending = None  # (s, bi, hts)
        for s, bi in sched:
            hts = phase_a(s, bi)
            if pending is not None:
                phase_b(pending[0], pending[1], pending[2], False)
            pending = (s, bi, hts)
        phase_b(pending[0], pending[1], pending[2], True)

    nc.compile()
    return nc


def _get(caps: tuple):
    if caps not in _CACHE:
        _CACHE[caps] = _build(caps)
    return _CACHE[caps]


def _assign_jobs(loads):
    """Pair the 2E half-H jobs (two per expert, load L_e each) big-with-small
    across E cores. Returns (caps, assign) where assign[core][slot] =
    (expert, half)."""
    jobs = [(e, h) for e in range(E) for h in range(2)]
    jobs.sort(key=lambda j: loads[j[0]], reverse=True)
    n = len(jobs)
    pairs = [(jobs[k], jobs[n - 1 - k]) for k in range(E)]
    c1 = max(loads[j[0]] for j, _ in pairs)
    c2 = max(loads[j[0]] for _, j in pairs)
    cap = lambda c: max(128, int(np.ceil(c / 128) * 128))
    return (cap(c1), cap(c2)), pairs


def _pack_x(x2d_bf, rows_e, C, blocks):
    """x columns for one job: per-block [128, KA, tb] partition-major bf16."""
    xt = np.zeros((D, C), ml_dtypes.bfloat16)
    xt[:, : len(rows_e)] = x2d_bf[rows_e].T
    xpm = xt.reshape(KA, 128, C).transpose(1, 0, 2)  # [128, KA, C]
    return [
        np.ascontiguousarray(xpm[:, :, t0 : t0 + tb]) for t0, tb in blocks
    ]


def _prepare_core_inputs(x2d, w1, w2, w3, rows, cw_e, caps, pairs):
    bf = ml_dtypes.bfloat16
    x2d_bf = x2d.astype(bf)
    blocks_s = [_blocks_for(C) for C in caps]
    in_maps = []
    for core in range(E):
        m = {}
        for s, (e, h) in enumerate(pairs[core]):
            C = caps[s]
            hsl = slice(h * HH, (h + 1) * HH)
            for b, xb in enumerate(_pack_x(x2d_bf, rows[e], C, blocks_s[s])):
                m[f"xtp{s}_{b}"] = xb
            # [D, HH] -> [128, KA, HH] partition-major
            w1pm = w1[e][hsl].T.astype(bf).reshape(KA, 128, HH).transpose(1, 0, 2)
            w3pm = w3[e][hsl].T.astype(bf).reshape(KA, 128, HH).transpose(1, 0, 2)
            # [HH, D] -> [128, KM, D] partition-major
            w2pm = w2[e][:, hsl].T.astype(bf).reshape(KM, 128, D).transpose(1, 0, 2)
            m0 = 0
            for p, sz in enumerate(PIECES):
                sl = slice(m0 * 128, (m0 + sz) * 128)
                m[f"w1p{s}_{p}"] = np.ascontiguousarray(w1pm[:, :, sl])
                m[f"w3p{s}_{p}"] = np.ascontiguousarray(w3pm[:, :, sl])
                m0 += sz
            for dh in range(2):
                m[f"w2p{s}_{dh}"] = np.ascontiguousarray(
                    w2pm[:, :, dh * 512 : (dh + 1) * 512]
                )
            cwt = np.zeros((C,), np.float32)
            cwt[: len(rows[e])] = cw_e[e]
            m[f"cwt{s}"] = np.ascontiguousarray(cwt.reshape(C // 128, 128).T)
        in_maps.append(m)
    return in_maps


def run(inputs: dict, trace: bool = False, trace_cores=None):
    """Core implementation; returns (output, BassKernelResults)."""
    x = np.asarray(inputs["x"])
    router_w = np.asarray(inputs["router_w"], np.float32)
    w1 = np.asarray(inputs["w1"], np.float32)
    w2 = np.asarray(inputs["w2"], np.float32)
    w3 = np.asarray(inputs["w3"], np.float32)

    B, S, _ = x.shape
    assert x.shape[-1] == D and router_w.shape == (E, D), (x.shape, router_w.shape)
    assert w1.shape == (E, H, D) and w3.shape == (E, H, D) and w2.shape == (E, D, H)
    x2d = np.ascontiguousarray(x.reshape(-1, D).astype(np.float32))

    rows, cw_e, top2, slot = _route(x2d, router_w)
    loads = [len(r) for r in rows]
    caps, pairs = _assign_jobs(loads)

    nc = _get(caps)
    in_maps = _prepare_core_inputs(x2d, w1, w2, w3, rows, cw_e, caps, pairs)
    res = run_bass_kernel_spmd(
        nc,
        in_maps,
        list(range(E)),
        trace=trace,
        trace_cores=trace_cores,
    )

    # sum the two half-H partials per expert (both already cw-scaled)
    Cmax = max(caps)
    Y = np.zeros((E, Cmax, D), np.float32)
    for core in range(E):
        for s, (e, h) in enumerate(pairs[core]):
            Y[e, : caps[s]] += res.results[core][f"y{s}"]
    Yf = Y.reshape(E * Cmax, D)
    fi = top2.astype(np.int64) * Cmax + slot  # [T, 2]
    out = Yf[fi[:, 0]] + Yf[fi[:, 1]]
    return out.reshape(B, S, D).astype(x.dtype), res


def kernel(**inputs) -> np.ndarray:
    out, _ = run(inputs, trace=False)
    return out


# revision 15
# speedup vs baseline: 1.0067x; 1.0067x over previous
"""Mixture-of-Experts (8 experts, top-2, D=1024, H=2048, T=8192) on 8 trn2 cores.

Strategy: expert-parallel with host-side routing, load-balanced by splitting
each expert's FFN along the hidden dim into two half-H jobs (16 jobs total).
  - Router (tiny: [T,D]@[D,E]) runs on host in float64; top-2 selection was
    verified to match fp32 jax (cpu + neuron) selection for this problem size.
  - Each job (expert e, half h) computes, for the tokens routed to e:
        h1T = w1[e,hH].T-chunks @ xT    [H/2, C]
        hT  = silu(h1T) * h3T           [H/2, C]  (bf16)
        yT  = (hT.T chunks) @ w2[e,:,hH].T  -> partial y [C, D], cw-scaled
    The two halves' partial y are summed on the host (linear in cw).
  - Jobs are sorted by token load and paired big-with-small onto the 8
    cores, so per-core capacity is J1+J2 ~ 2*mean instead of 2*max.
  - Activations flow in transposed (feature-major) layout so the kernel
    needs no on-device transposes; phase B puts tokens on partitions so the
    per-token combine weight is a per-partition tensor_scalar op.
"""

import sys
import types
from contextlib import ExitStack

import ml_dtypes
import numpy as np

import concourse.bass as bass
import concourse.tile as tile
from concourse import bacc, mybir
from concourse.bass_utils import run_bass_kernel_spmd


def install_axon_hooks_shim():
    """The container's antenv stub lacks axon_hooks, which
    run_bass_kernel_spmd imports whenever tracing is requested (including
    via the BASS_TRACE env var). Recreate it and register the NTFF
    profiling hook if the axon PJRT .so is present."""
    try:
        import antenv
    except ImportError:
        return False
    if "antenv.axon_hooks" in sys.modules:
        return sys.modules["antenv.axon_hooks"]._hook is not None
    mod = types.ModuleType("antenv.axon_hooks")
    mod._hook = None
    mod.set_axon_ntff_profile_hook = lambda h: setattr(mod, "_hook", h)
    mod.get_axon_ntff_profile_hook = lambda: mod._hook
    sys.modules["antenv.axon_hooks"] = mod
    antenv.axon_hooks = mod
    try:
        from trn_agent_boot.trn_boot import _ntff_profile_via_ctypes

        mod.set_axon_ntff_profile_hook(
            _ntff_profile_via_ctypes("/opt/axon/libaxon_pjrt.so")
        )
    except Exception:
        pass
    return mod._hook is not None


install_axon_hooks_shim()

E = 8  # experts
D = 1024
H = 2048
HH = H // 2  # hidden half per job
TOP_K = 2
KA = D // 128  # contraction chunks for matmul 1
KM = HH // 128  # contraction chunks for matmul 2 (half hidden)

BF16 = mybir.dt.bfloat16
F32 = mybir.dt.float32

# w1/w3 half-H piece sizes in m-chunks (small first so early matmuls start
# early)
PIECES = (1, 1, 2, 4)

_CACHE: dict[tuple, object] = {}


def _route(x2d: np.ndarray, router_w: np.ndarray):
    """Float64 router. Returns per-expert token lists, per-expert combine
    weights, and for each token its (expert, slot-in-expert-batch) pairs."""
    T = x2d.shape[0]
    logits = x2d.astype(np.float64) @ router_w.astype(np.float64).T  # [T, E]
    order = np.argsort(-logits, axis=1, kind="stable")
    top2 = order[:, :TOP_K]  # [T, 2]
    lt = np.take_along_axis(logits, top2, axis=1)
    m = lt.max(axis=1, keepdims=True)
    ex = np.exp(lt - m)
    cw = (ex / ex.sum(axis=1, keepdims=True)).astype(np.float32)  # [T, 2]

    rows = []  # rows[e]: token ids routed to expert e (ascending)
    cw_e = []  # cw_e[e]: combine weight per routed token
    slot = np.empty((T, TOP_K), np.int64)  # slot[t, k]: row of t in expert batch
    for e in range(E):
        r = np.where((top2[:, 0] == e) | (top2[:, 1] == e))[0]
        k = np.where(top2[r, 0] == e, 0, 1)
        rows.append(r)
        cw_e.append(cw[r, k])
        slot[r, k] = np.arange(len(r))
    return rows, cw_e, top2, slot


def _blocks_for(C):
    """Token blocks for a slot of capacity C (a multiple of 128). Blocks are
    3 or 4 subtiles (384/512) so phase-A matmul free dims stay well above the
    ~240-cycle LDWEIGHTS latency."""
    n = C // 128
    sizes = []
    while n > 0:
        take = 3 if n == 5 else min(4, n)
        sizes.append(take * 128)
        n -= take
    blocks, t0 = [], 0
    for sz in sizes:
        blocks.append((t0, sz))
        t0 += sz
    return blocks


def _build(caps: tuple):
    """Build + compile the per-core Bass program for slot capacities `caps`
    (each a multiple of 128).

    All inputs are shipped pre-arranged in SBUF partition-major layout so
    every DMA is ~128 large contiguous descriptors (descriptor rate, not
    bandwidth, limits small strided transfers)."""
    nslots = len(caps)
    for C in caps:
        assert C % 128 == 0
    blocks_s = [_blocks_for(C) for C in caps]

    nc = bacc.Bacc("TRN2", target_bir_lowering=False, debug=False)

    xtps, w1ps, w3ps, w2ps, cwts, ys, yvs = [], [], [], [], [], [], []
    for s, C in enumerate(caps):
        nsub = C // 128
        # x per token block, exact-size so the DMA is 128 contiguous rows
        xtps.append([
            nc.declare_dram_parameter(f"xtp{s}_{b}", [128, KA, tb], BF16, isOutput=False)
            for b, (t0, tb) in enumerate(blocks_s[s])
        ])
        w1ps.append([
            nc.declare_dram_parameter(f"w1p{s}_{p}", [128, KA, sz * 128], BF16, isOutput=False)
            for p, sz in enumerate(PIECES)
        ])
        w3ps.append([
            nc.declare_dram_parameter(f"w3p{s}_{p}", [128, KA, sz * 128], BF16, isOutput=False)
            for p, sz in enumerate(PIECES)
        ])
        w2ps.append([
            nc.declare_dram_parameter(f"w2p{s}_{dh}", [128, KM, 512], BF16, isOutput=False)
            for dh in range(2)
        ])
        cwts.append(nc.declare_dram_parameter(f"cwt{s}", [128, nsub], F32, isOutput=False))
        y = nc.declare_dram_parameter(f"y{s}", [C, D], F32, isOutput=True)
        ys.append(y)
        yvs.append(y.rearrange("(n p) d -> n p d", p=128))  # [nsub, 128, D]

    with ExitStack() as ctx:
        tc = ctx.enter_context(tile.TileContext(nc))
        wpool = ctx.enter_context(tc.tile_pool(name="weights", bufs=1))
        xpool = ctx.enter_context(tc.tile_pool(name="x", bufs=6))
        hpool = ctx.enter_context(tc.tile_pool(name="h", bufs=2))
        spool = ctx.enter_context(tc.tile_pool(name="s", bufs=3))
        ypool = ctx.enter_context(tc.tile_pool(name="y", bufs=4))
        ppool = ctx.enter_context(tc.tile_pool(name="psum", bufs=2, space="PSUM"))

        def xts_load(s, bi):
            xa = xpool.tile([128, KA, blocks_s[s][bi][1]], BF16, tag="xts")
            nc.sync.dma_start(xa[:], xtps[s][bi][:])
            return xa

        def wpiece_load(kind, src, s, p, sz):
            t = wpool.tile([128, KA, sz * 128], BF16, tag=f"{kind}{s}_{p}")
            nc.sync.dma_start(t[:], src[:])
            return t

        # DMA issue order is the DMA *execution* order; the aggregate rate is
        # ~0.36 MB/us, so interleave slot0's x blocks with its weight pieces
        # to keep the PE fed during the first ~50us.
        xts_tiles = {}
        w1p_s, w3p_s, w2p_s, cws_s = [], [], [], []

        # each dma_start costs ~0.6us of Sync-engine enqueue time, so the ramp
        # uses few, large transfers. Order = DMA execution order: the m=0
        # w1/w3 piece, x block0, then the remaining w1/w3 pieces (phase A
        # consumes them within ~15us; the later x blocks can wait)
        xts_tiles[(0, 0)] = xts_load(0, 0)
        w1p0, w3p0 = [], []
        for p, sz in enumerate(PIECES):
            t1 = wpiece_load("w1s", w1ps[0][p], 0, p, sz)
            t3 = wpiece_load("w3s", w3ps[0][p], 0, p, sz)
            for i in range(sz):
                w1p0.append((t1, i))
                w3p0.append((t3, i))
        for bi in range(1, len(blocks_s[0])):
            xts_tiles[(0, bi)] = xts_load(0, bi)
        w1p_s.append(w1p0)
        w3p_s.append(w3p0)
        row = []
        for dh in range(2):
            t2 = wpool.tile([128, KM, 512], BF16, tag=f"w2s0_{dh}")
            nc.sync.dma_start(t2[:], w2ps[0][dh][:])
            row.append(t2)
        w2p_s.append(row)
        cws = wpool.tile([128, caps[0] // 128], F32, tag="cws0")
        nc.sync.dma_start(cws[:], cwts[0][:])
        cws_s.append(cws)

        for s in range(1, nslots):
            w1p, w3p = [], []
            for p, sz in enumerate(PIECES):
                t1 = wpiece_load("w1s", w1ps[s][p], s, p, sz)
                t3 = wpiece_load("w3s", w3ps[s][p], s, p, sz)
                for i in range(sz):
                    w1p.append((t1, i))
                    w3p.append((t3, i))
            w1p_s.append(w1p)
            w3p_s.append(w3p)
            row = []
            for dh in range(2):
                t2 = wpool.tile([128, KM, 512], BF16, tag=f"w2s{s}_{dh}")
                nc.sync.dma_start(t2[:], w2ps[s][dh][:])
                row.append(t2)
            w2p_s.append(row)
            cws = wpool.tile([128, caps[s] // 128], F32, tag=f"cws{s}")
            nc.sync.dma_start(cws[:], cwts[s][:])
            cws_s.append(cws)

        def phase_a(s, bi):
            t0, tb = blocks_s[s][bi]
            xts = xts_tiles.pop((s, bi), None)
            if xts is None:
                xts = xts_load(s, bi)
            w1p, w3p = w1p_s[s], w3p_s[s]
            hts = hpool.tile([128, KM, tb], BF16, tag="hts")
            for m in range(KM):
                w1s, o1 = w1p[m]
                w3s, o3 = w3p[m]
                ph1 = ppool.tile([128, tb], F32, tag="ph1")
                for a in range(KA):
                    nc.tensor.matmul(
                        ph1[:],
                        w1s[:, a, bass.ts(o1, 128)],
                        xts[:, a, :],
                        start=(a == 0),
                        stop=(a == KA - 1),
                    )
                ph3 = ppool.tile([128, tb], F32, tag="ph3")
                for a in range(KA):
                    nc.tensor.matmul(
                        ph3[:],
                        w3s[:, a, bass.ts(o3, 128)],
                        xts[:, a, :],
                        start=(a == 0),
                        stop=(a == KA - 1),
                    )
                sil = spool.tile([128, tb], BF16, tag="sil")
                nc.scalar.activation(
                    sil[:], ph1[:], mybir.ActivationFunctionType.Silu
                )
                nc.vector.tensor_mul(hts[:, m, :], sil[:], ph3[:])
            return hts

        def phase_b(s, bi, hts, is_last):
            t0, tb = blocks_s[s][bi]
            w2p, cws = w2p_s[s], cws_s[s]
            for n in range(tb // 128):
                nsl = bass.ts(n, 128)
                gn = t0 // 128 + n  # global subtile index
                split = is_last and n == tb // 128 - 1
                if split:
                    # final subtile: emit progressively narrower pieces, each
                    # DMA'd as soon as ready, so the tail after the last
                    # matmul is a single small scale+DMA
                    pieces = [(0, 512, "py0", "ys0"), (512, 384, "py1", "ys1"),
                              (896, 128, "py1", "ys1")]
                    for d0, dw, ptag, ytag in pieces:
                        py = ppool.tile([128, dw], F32, tag=ptag)
                        for m in range(KM):
                            nc.tensor.matmul(
                                py[:],
                                hts[:, m, nsl],
                                w2p[d0 // 512][:, m, d0 % 512 : d0 % 512 + dw],
                                start=(m == 0),
                                stop=(m == KM - 1),
                            )
                        yt = ypool.tile([128, dw], F32, tag=ytag)
                        nc.vector.tensor_scalar_mul(yt[:], py[:], cws[:, gn : gn + 1])
                        nc.sync.dma_start(yvs[s][gn][:, d0 : d0 + dw], yt[:])
                    continue
                py0 = ppool.tile([128, 512], F32, tag="py0")
                py1 = ppool.tile([128, 512], F32, tag="py1")
                for m in range(KM):
                    nc.tensor.matmul(
                        py0[:],
                        hts[:, m, nsl],
                        w2p[0][:, m, :],
                        start=(m == 0),
                        stop=(m == KM - 1),
                    )
                    nc.tensor.matmul(
                        py1[:],
                        hts[:, m, nsl],
                        w2p[1][:, m, :],
                        start=(m == 0),
                        stop=(m == KM - 1),
                    )
                ys0 = ypool.tile([128, 512], F32, tag="ys0")
                nc.vector.tensor_scalar_mul(ys0[:], py0[:], cws[:, gn : gn + 1])
                nc.sync.dma_start(yvs[s][gn][:, 0:512], ys0[:])
                ys1 = ypool.tile([128, 512], F32, tag="ys1")
                nc.vector.tensor_scalar_mul(ys1[:], py1[:], cws[:, gn : gn + 1])
                nc.sync.dma_start(yvs[s][gn][:, 512:1024], ys1[:])

        # software pipeline: A(g+1) is issued before B(g) so the PE never
        # waits on w2 / output machinery during the DMA-heavy ramp
        sched = [(s, bi) for s in range(nslots) for bi in range(len(blocks_s[s]))]
        pending = None  # (s, bi, hts)
        for s, bi in sched:
            hts = phase_a(s, bi)
            if pending is not None:
                phase_b(pending[0], pending[1], pending[2], False)
            pending = (s, bi, hts)
        phase_b(pending[0], pending[1], pending[2], True)

    nc.compile()
    return nc


def _get(caps: tuple):
    if caps not in _CACHE:
        _CACHE[caps] = _build(caps)
    return _CACHE[caps]


def _assign_jobs(loads):
    """Pair the 2E half-H jobs (two per expert, load L_e each) big-with-small
    across E cores. Returns (caps, assign) where assign[core][slot] =
    (expert, half)."""
    jobs = [(e, h) for e in range(E) for h in range(2)]
    jobs.sort(key=lambda j: loads[j[0]], reverse=True)
    n = len(jobs)
    pairs = [(jobs[k], jobs[n - 1 - k]) for k in range(E)]
    c1 = max(loads[j[0]] for j, _ in pairs)
    c2 = max(loads[j[0]] for _, j in pairs)
    cap = lambda c: max(128, int(np.ceil(c / 128) * 128))
    return (cap(c1), cap(c2)), pairs


def _pack_x(x2d_bf, rows_e, C, blocks):
    """x columns for one job: per-block [128, KA, tb] partition-major bf16."""
    xt = np.zeros((D, C), ml_dtypes.bfloat16)
    xt[:, : len(rows_e)] = x2d_bf[rows_e].T
    xpm = xt.reshape(KA, 128, C).transpose(1, 0, 2)  # [128, KA, C]
    return [
        np.ascontiguousarray(xpm[:, :, t0 : t0 + tb]) for t0, tb in blocks
    ]


def _prepare_core_inputs(x2d, w1, w2, w3, rows, cw_e, caps, pairs):
    bf = ml_dtypes.bfloat16
    x2d_bf = x2d.astype(bf)
    blocks_s = [_blocks_for(C) for C in caps]
    in_maps = []
    for core in range(E):
        m = {}
        for s, (e, h) in enumerate(pairs[core]):
            C = caps[s]
            hsl = slice(h * HH, (h + 1) * HH)
            for b, xb in enumerate(_pack_x(x2d_bf, rows[e], C, blocks_s[s])):
                m[f"xtp{s}_{b}"] = xb
            # [D, HH] -> [128, KA, HH] partition-major
            w1pm = w1[e][hsl].T.astype(bf).reshape(KA, 128, HH).transpose(1, 0, 2)
            w3pm = w3[e][hsl].T.astype(bf).reshape(KA, 128, HH).transpose(1, 0, 2)
            # [HH, D] -> [128, KM, D] partition-major
            w2pm = w2[e][:, hsl].T.astype(bf).reshape(KM, 128, D).transpose(1, 0, 2)
            m0 = 0
            for p, sz in enumerate(PIECES):
                sl = slice(m0 * 128, (m0 + sz) * 128)
                m[f"w1p{s}_{p}"] = np.ascontiguousarray(w1pm[:, :, sl])
                m[f"w3p{s}_{p}"] = np.ascontiguousarray(w3pm[:, :, sl])
                m0 += sz
            for dh in range(2):
                m[f"w2p{s}_{dh}"] = np.ascontiguousarray(
                    w2pm[:, :, dh * 512 : (dh + 1) * 512]
                )
            cwt = np.zeros((C,), np.float32)
            cwt[: len(rows[e])] = cw_e[e]
            m[f"cwt{s}"] = np.ascontiguousarray(cwt.reshape(C // 128, 128).T)
        in_maps.append(m)
    return in_maps


def run(inputs: dict, trace: bool = False, trace_cores=None):
    """Core implementation; returns (output, BassKernelResults)."""
    x = np.asarray(inputs["x"])
    router_w = np.asarray(inputs["router_w"], np.float32)
    w1 = np.asarray(inputs["w1"], np.float32)
    w2 = np.asarray(inputs["w2"], np.float32)
    w3 = np.asarray(inputs["w3"], np.float32)

    B, S, _ = x.shape
    assert x.shape[-1] == D and router_w.shape == (E, D), (x.shape, router_w.shape)
    assert w1.shape == (E, H, D) and w3.shape == (E, H, D) and w2.shape == (E, D, H)
    x2d = np.ascontiguousarray(x.reshape(-1, D).astype(np.float32))

    rows, cw_e, top2, slot = _route(x2d, router_w)
    loads = [len(r) for r in rows]
    caps, pairs = _assign_jobs(loads)

    nc = _get(caps)
    in_maps = _prepare_core_inputs(x2d, w1, w2, w3, rows, cw_e, caps, pairs)
    res = run_bass_kernel_spmd(
        nc,
        in_maps,
        list(range(E)),
        trace=trace,
        trace_cores=trace_cores,
    )

    # sum the two half-H partials per expert (both already cw-scaled)
    Cmax = max(caps)
    Y = np.zeros((E, Cmax, D), np.float32)
    for core in range(E):
        for s, (e, h) in enumerate(pairs[core]):
            Y[e, : caps[s]] += res.results[core][f"y{s}"]
    Yf = Y.reshape(E * Cmax, D)
    fi = top2.astype(np.int64) * Cmax + slot  # [T, 2]
    out = Yf[fi[:, 0]] + Yf[fi[:, 1]]
    return out.reshape(B, S, D).astype(x.dtype), res


def kernel(**inputs) -> np.ndarray:
    out, _ = run(inputs, trace=False)
    return out
